# revision 28
# baseline (speedup 1.0000x reference)
"""Trainium2 Bass kernel for nn_Encoder_78795470012907.

Encoder layer: per-head Q/K/V projections, scores = QK^T/sqrt(dk),
double softmax (over batch axis, then over key axis), Z = pV, concat
heads, output projection. S=2048, B=4, D=512, H=8, dk=dv=64.

Sharding: head-parallel over 8 cores (core h owns head h) for the
attention; four quarter-AllToAlls (one per s-chunk of 512) re-shard by
token for the output projection. Core c owns tokens
{(b=c//2, s = q*512 + (c%2)*256 + [0,256)) : q in 0..3} and emits them
as 4 quarters of 256 rows (host just scatters).

Layout notes (per core):
 - tokens are b-major: tok = b*2048 + s.
 - X is fed pre-transposed AND pre-cast from host as XT [D, NTOK] bf16.
 - V bias folds out (softmax rows sum to 1): bo' = bO + bv_cat @ WO.
 - phase A PSUM evacuation runs on the otherwise-idle DVE
   (tensor_scalar_add applies the Q/K bias per-partition); ACT does
   nothing in phase A so phase B's exp pipeline is the only ACT work.
 - scores are computed transposed ([t, s] tiles); the two batches of a
   b-pair run as concurrent row-tiled matmuls.
 - softmax over b: e=exp(s/8); D1 via ONE strided tensor_reduce over
   the b axis; r=1/D1 fast-reciprocal; p1=e*r.
 - softmax over t rides the Z matmul via a ones-column appended to V
   (row 64 of the Z psum accumulates D2 = sum_t exp(p1)).
 - sc boundary = 8 plain DVE copies (z rows 0-63 -> zt bf16 releasing
   one PSUM bank per b, D2 row -> dz bf16).  NO reciprocal, broadcast
   or multiply sits in the boundary dependency chain (v2 lesson: any
   broadcast there stalls the next sc's Z matmuls ~15us).
 - the a2a ships [65, 256] chunks (z + raw D2 row, bf16).  The dest
   normalizes at the TAIL, where the vector queue is empty: gpsimd
   partition_broadcast of the D2 row, cast-up, fast reciprocal,
   multiply -- per quarter, overlapped with the last collective.
"""

from contextlib import ExitStack

import numpy as np
import ml_dtypes

import concourse.bass as bass
import concourse.tile as tile
from concourse import bacc, mybir
from concourse.bass_utils import run_bass_kernel_spmd

S, B, D = 2048, 4, 512
H, DK, DV = 8, 64, 64
N_CORES = 8
NTOK = S * B          # 8192 tokens, b-major
TOKC = NTOK // N_CORES  # 1024 tokens per core for the output slice
SC = 512              # s-chunk (columns of a scores^T tile)
TC = 128              # t-chunk (partitions of a scores^T tile)
N_SC = S // SC        # 4
N_TC = S // TC        # 16
QT = 256              # tokens per (core, quarter)

F32 = mybir.dt.float32
BF16 = mybir.dt.bfloat16
AF = mybir.ActivationFunctionType
ALU = mybir.AluOpType
AX = mybir.AxisListType
BF = ml_dtypes.bfloat16


def build_kernel():
    nc = bacc.Bacc(num_devices=N_CORES)

    xt_d = nc.dram_tensor("xt", [D, NTOK], BF16, kind="ExternalInput")
    wqk_d = nc.dram_tensor("wqk", [D, 128], BF16, kind="ExternalInput")
    bqk_d = nc.dram_tensor("bqk", [128, 1], F32, kind="ExternalInput")
    wv_d = nc.dram_tensor("wv", [D, DV], BF16, kind="ExternalInput")
    wo_d = nc.dram_tensor("wo", [D, D], BF16, kind="ExternalInput")
    bo_d = nc.dram_tensor("bo", [1, D], BF16, kind="ExternalInput")
    out_d = nc.dram_tensor("out", [TOKC, D], BF16, kind="ExternalOutput")

    with tile.TileContext(nc) as tc, ExitStack() as ctx:
        pp = ctx.enter_context(tc.tile_pool(name="persist", bufs=1))
        dram = ctx.enter_context(tc.tile_pool(name="dram", bufs=1, space="DRAM"))

        # ---- persistent SBUF ----
        qt = [pp.tile([128, S], BF16, tag=f"qt{p}", name=f"qt{p}") for p in range(2)]
        kt = [pp.tile([128, S], BF16, tag=f"kt{p}", name=f"kt{p}") for p in range(2)]
        vt = pp.tile([128, 64 * 65], BF16, tag="vt", name="vt")
        # Z^T unnormalized [64, NTOK] + D2 row [1, NTOK], bf16
        zt = pp.tile([64, NTOK], BF16, tag="zt", name="zt")
        dz = pp.tile([1, NTOK], BF16, tag="dz", name="dz")

        wqk = [pp.tile([128, 128], BF16, tag=f"wqk{i}", name=f"wqk{i}") for i in range(4)]
        wv = [pp.tile([128, DV], BF16, tag=f"wv{i}", name=f"wv{i}") for i in range(4)]
        wo = [pp.tile([128, D], BF16, tag=f"wo{i}", name=f"wo{i}") for i in range(4)]
        bqk = pp.tile([128, 1], F32, tag="bqk", name="bqk")
        bo = pp.tile([1, D], BF16, tag="bo", name="bo")

        for i in range(4):
            nc.sync.dma_start(wqk[i][:], wqk_d[i * 128:(i + 1) * 128, :])
        nc.sync.dma_start(bqk[:], bqk_d[:])
        for i in range(4):
            nc.gpsimd.dma_start(wv[i][:], wv_d[i * 128:(i + 1) * 128, :])
            nc.gpsimd.dma_start(wo[i][:], wo_d[i * 128:(i + 1) * 128, :])
        nc.gpsimd.dma_start(bo[:], bo_d[:])

        onesf = pp.tile([128, 128], F32, tag="onesf", name="onesf")
        nc.vector.memset(onesf[:], 1.0)
        ones_bf = pp.tile([1, 128], BF16, tag="ones_bf", name="ones_bf")
        nc.vector.tensor_copy(ones_bf[:], onesf[0:1, :])
        vt_ones = vt[:].rearrange("p (n c) -> p n c", c=65)[:, :, 64:65]
        nc.vector.tensor_copy(vt_ones, onesf[:, 0:64, None])
        # warm the ACT exp table set and the gpsimd broadcast library
        warm = pp.tile([1, 1], BF16, tag="warm", name="warm")
        nc.scalar.activation(warm[:], onesf[0:1, 0:1], AF.Exp)
        warmb = pp.tile([2, 16], BF16, tag="warmb", name="warmb")
        nc.gpsimd.partition_broadcast(warmb[:], ones_bf[:, 0:16])

        # ================= Phase A: projections =================
        # PE-bound; PSUM evacuation (+ Q/K bias + bf16 cast) rides the
        # otherwise-idle DVE so ACT stays free for phase B's exps.
        with (
            tc.tile_pool(name="xtp", bufs=2) as xp,
            tc.tile_pool(name="psA", bufs=2, space="PSUM") as psA,
        ):
            for ssub in range(4):
                xtile = [xp.tile([128, 4, 512], BF16, tag=f"xt{i}",
                                 name=f"xtile{i}") for i in range(4)]
                xsrc = xt_d[:].rearrange("d (b ss s) -> d b ss s", b=4, ss=4)
                for i in range(4):
                    nc.sync.dma_start(
                        xtile[i][:],
                        xsrc[i * 128:(i + 1) * 128, :, ssub, :],
                    )
                for b in range(4):
                    pair, row = b // 2, (b % 2) * 64
                    pqk = psA.tile([128, 512], F32, tag="pqk", name="pqk",
                                   bufs=3)
                    for i in range(4):
                        nc.tensor.matmul(pqk[:], wqk[i][:], xtile[i][:, b, :],
                                         start=(i == 0), stop=(i == 3))
                    scol = ssub * 512
                    nc.vector.tensor_scalar_add(
                        qt[pair][row:row + 64, scol:scol + 512],
                        pqk[0:64, :], bqk[0:64, :])
                    nc.vector.tensor_scalar_add(
                        kt[pair][row:row + 64, scol:scol + 512],
                        pqk[64:128, :], bqk[64:128, :])
                    pv = psA.tile([128, 4 * DV], F32, tag="pv", name="pv",
                                  bufs=2)
                    for sub in range(4):
                        for i in range(4):
                            nc.tensor.matmul(
                                pv[:, sub * DV:(sub + 1) * DV],
                                xtile[i][:, b, sub * 128:(sub + 1) * 128],
                                wv[i][:], start=(i == 0), stop=(i == 3))
                    tbase = (b * 4 + ssub) * 4
                    nc.vector.tensor_copy(
                        vt[:].rearrange("p (n c) -> p n c", c=65)
                        [:, tbase:tbase + 4, 0:64],
                        pv[:].rearrange("p (n c) -> p n c", c=DV))

        # ================= Phase B: attention =================
        # per-quarter a2a staging: each quarter q ships, per dest core
        # c=(b,half), [65, 256]: 64 unnormalized z rows + the raw D2 row
        # for tokens (b, s in q*512 + half*256 + [0,256)).
        a2a_in_h = [dram.tile([N_CORES * 65, QT], BF16, tag=f"a2a_in{q}",
                              name=f"a2a_in{q}") for q in range(4)]
        a2a_out_h = [dram.tile([N_CORES * 65, QT], BF16, tag=f"a2a_out{q}",
                               name=f"a2a_out{q}") for q in range(4)]

        with (
            tc.tile_pool(name="wb", bufs=2) as wb,
        ):
            with tc.tile_pool(name="psB", bufs=1, space="PSUM") as psB:
                NB = N_SC * N_TC
                pipe = {}

                def softmax_b(g):
                    sc, t = g // N_TC, g % N_TC
                    scp = psB.tile([128, 4 * SC], F32, tag="scp", name="scp")
                    for pair in range(2):
                        for half in range(2):
                            b = pair * 2 + half
                            row = half * 64
                            nc.tensor.matmul(
                                scp[:, b * SC:(b + 1) * SC],
                                kt[pair][row:row + 64, t * TC:(t + 1) * TC],
                                qt[pair][row:row + 64, sc * SC:(sc + 1) * SC],
                                start=True, stop=True,
                            )
                    e = wb.tile([128, 4 * SC], BF16, tag="e", name="e", bufs=4)
                    nc.scalar.activation(e[:], scp[:], AF.Exp, scale=0.125)
                    t01 = wb.tile([128, 2 * SC], BF16, tag="t01", name="t01",
                                  bufs=2)
                    nc.vector.tensor_add(t01[:], e[:, 0:2 * SC],
                                         e[:, 2 * SC:4 * SC])
                    ddf = wb.tile([128, SC], F32, tag="ddf", name="ddf", bufs=2)
                    nc.vector.tensor_add(ddf[:], t01[:, 0:SC], t01[:, SC:2 * SC])
                    rf = wb.tile([128, SC], F32, tag="rf", name="rf", bufs=2)
                    nc.vector.reciprocal_approx_fast(rf[:], ddf[:])
                    rr = wb.tile([128, SC], BF16, tag="rr", name="rr", bufs=2)
                    nc.vector.tensor_copy(rr[:], rf[:])
                    p1 = wb.tile([128, 4 * SC], BF16, tag="p1", name="p1",
                                 bufs=4)
                    nc.vector.tensor_mul(
                        p1[:].rearrange("p (b s) -> p b s", b=4),
                        e[:].rearrange("p (b s) -> p b s", b=4),
                        rr[:, None, :].broadcast_to([128, 4, SC]),
                    )
                    pipe[g] = p1

                def exp2_and_z(g, zacc):
                    t = g % N_TC
                    p1 = pipe.pop(g)
                    q = wb.tile([128, 4 * SC], BF16, tag="q", name="q", bufs=4)
                    nc.scalar.activation(q[:], p1[:], AF.Exp)
                    for b in range(4):
                        tci = b * 16 + t
                        nc.tensor.matmul(
                            zacc[:, b * SC:(b + 1) * SC],
                            vt[:, tci * 65:(tci + 1) * 65],
                            q[:, b * SC:(b + 1) * SC],
                            start=(t == 0), stop=(t == N_TC - 1),
                        )

                def sc_boundary(sc_done, za):
                    # plain evac only: z rows -> zt (releases one PSUM bank
                    # per b), D2 row -> dz.  Normalization happens on the
                    # destination core at the tail.
                    with tc.high_priority():
                        ztv = zt[:].rearrange("p (b s8) -> p b s8", b=4)
                        dzv = dz[:].rearrange("p (b s8) -> p b s8", b=4)
                        for b in range(4):
                            sl = slice(b * SC, (b + 1) * SC)
                            ssl = slice(sc_done * SC, (sc_done + 1) * SC)
                            # z rows evac on ACT (rebalance: DVE is the
                            # tighter engine in phase B), D2 row on DVE
                            nc.scalar.activation(
                                ztv[:, b, ssl], za[0:64, sl], AF.Identity)
                            nc.vector.tensor_copy(
                                dzv[:, b, ssl], za[64:65, sl])
                        # ship quarter sc_done: z rows (2 DMAs by half) +
                        # D2 row (1 DMA)
                        zsrc = zt[:].rearrange(
                            "p (b sc half u) -> p b sc half u",
                            b=4, sc=4, half=2)
                        dsrc = dz[:].rearrange(
                            "p (b sc half u) -> p b sc half u",
                            b=4, sc=4, half=2)
                        av = a2a_in_h[sc_done][:].rearrange(
                            "(b half p) u -> p b half u", b=4, half=2, p=65)
                        for half in range(2):
                            nc.sync.dma_start(
                                av[0:64, :, half, :],
                                zsrc[:, :, sc_done, half, :])
                            nc.sync.dma_start(
                                av[64:65, :, half, :],
                                dsrc[:, :, sc_done, half, :])
                        nc.gpsimd.collective_compute(
                            "AllToAll",
                            ALU.bypass,
                            replica_groups=[list(range(N_CORES))],
                            ins=[a2a_in_h[sc_done][:].opt()],
                            outs=[a2a_out_h[sc_done][:].opt()],
                        )

                zaccs = {}
                for g in range(NB + 2):
                    if g < NB:
                        if g % N_TC == 0:
                            zaccs[g // N_TC] = psB.tile([65, 4 * SC], F32,
                                                        tag="zacc", name="zacc")
                        softmax_b(g)
                    if g >= 2:
                        gz = g - 2
                        za = zaccs[gz // N_TC]
                        exp2_and_z(gz, za)
                        if gz % N_TC == N_TC - 1:
                            sc_boundary(gz // N_TC, za)

            # ---- Phase C: normalize + output projection, per quarter ----
            # The static scheduler models collective latency optimistically
            # and will happily slot collective-downstream ops into
            # mid-phase-B queue positions, where they head-of-line block
            # the engine queues (~20us each).  GATE each quarter's gathers
            # behind a tiny DVE memset emitted at the END of phase B so no
            # dest-side op can be scheduled before phase B drains.
            with (
                tc.tile_pool(name="psC", bufs=2, space="PSUM") as psC,
                tc.tile_pool(name="oc", bufs=1) as oc,
            ):
                zcs = [oc.tile([64, 8 * QT], BF16, tag=f"zc{q}",
                               name=f"zc{q}") for q in range(4)]
                dqs = [oc.tile([1, 8 * QT], BF16, tag=f"dq{q}",
                               name=f"dq{q}") for q in range(4)]
                for q in range(4):
                    nc.vector.memset(zcs[q][0:1, :], 0.0)
                    nc.vector.memset(dqs[q][:], 0.0)
                for q in range(4):
                    zc, dq = zcs[q], dqs[q]
                    src = a2a_out_h[q][:].rearrange(
                        "(h p) u -> p h u", p=65)
                    # gathers ride the sync queue (nothing queued behind
                    # them at the tail can matter)
                    nc.sync.dma_start(
                        zc[:].rearrange("p (h u) -> p h u", h=8),
                        src[0:64, :, :])
                    nc.sync.dma_start(
                        dq[:].rearrange("p (h u) -> p h u", h=8),
                        src[64:65, :, :])
                    # dest-side normalize: ONE base-0 partition_broadcast of
                    # the raw D2 row, cast-up on ACT, 1/x on DVE, then two
                    # strided equal-base multiplies into the head-pair
                    # stacked zcn layout phase C consumes.
                    rb = oc.tile([64, 8 * QT], BF16, tag="rb", name="rb",
                                 bufs=2)
                    nc.gpsimd.partition_broadcast(rb[:], dq[:])
                    rbf = oc.tile([64, 8 * QT], F32, tag="rbf", name="rbf",
                                  bufs=2)
                    nc.scalar.activation(rbf[:], rb[:], AF.Identity)
                    rbr = oc.tile([64, 8 * QT], F32, tag="rbr", name="rbr",
                                  bufs=2)
                    nc.vector.reciprocal_approx_fast(rbr[:], rbf[:])
                    zcn = oc.tile([128, 4 * QT], BF16, tag="zcn", name="zcn",
                                  bufs=2)
                    zcv = zc[:].rearrange("p (i hh u) -> p hh i u",
                                          i=4, hh=2)
                    rbv = rbr[:].rearrange("p (i hh u) -> p hh i u",
                                           i=4, hh=2)
                    for hh in range(2):
                        nc.vector.tensor_mul(
                            zcn[hh * 64:(hh + 1) * 64, :]
                            .rearrange("p (i u) -> p i u", i=4),
                            zcv[:, hh, :, :], rbv[:, hh, :, :])
                    for m in range(QT // 128):
                        po = psC.tile([128, D], F32, tag="po", name="po")
                        for i in range(4):
                            nc.tensor.matmul(po[:],
                                             zcn[:, i * QT + m * 128:i * QT + (m + 1) * 128],
                                             wo[i][:], start=(i == 0),
                                             stop=False)
                        nc.tensor.matmul(po[:], ones_bf[:], bo[:], start=False,
                                         stop=True)
                        ot = oc.tile([128, D], BF16, tag="ot", name="ot")
                        nc.scalar.activation(ot[:], po[:], AF.Identity)
                        row = q * QT + m * 128
                        # out DMAs follow their evacs on the scalar queue
                        nc.scalar.dma_start(out_d[row:row + 128, :], ot[:])

    nc.compile()
    return nc


_NC_CACHE = None


def _get_nc():
    global _NC_CACHE
    if _NC_CACHE is None:
        _NC_CACHE = build_kernel()
    return _NC_CACHE


def kernel(X, WQ, bQ, WK, bK, WV, bV, WO, bO, _trace=False, _trace_kwargs=None):
    """Full inputs in, full output out. Shards internally across 8 cores."""
    X = np.asarray(X, dtype=np.float32)
    WQ, bQ = np.asarray(WQ, np.float32), np.asarray(bQ, np.float32)
    WK, bK = np.asarray(WK, np.float32), np.asarray(bK, np.float32)
    WV, bV = np.asarray(WV, np.float32), np.asarray(bV, np.float32)
    WO, bO = np.asarray(WO, np.float32), np.asarray(bO, np.float32)
    xt = np.ascontiguousarray(
        X.transpose(2, 1, 0).reshape(D, NTOK)).astype(BF)
    # fold the V bias through the output projection (softmax rows sum to 1)
    bo_eff = (bO + bV.reshape(1, H * DV) @ WO).astype(np.float32)
    in_maps = []
    for h in range(N_CORES):
        wqk = np.ascontiguousarray(
            np.concatenate([WQ[h], WK[h]], axis=1)).astype(BF)
        bqk = np.ascontiguousarray(
            np.concatenate([bQ[h], bK[h]])[:, None], dtype=np.float32)
        in_maps.append({
            "xt": xt,
            "wqk": wqk,
            "bqk": bqk,
            "wv": np.ascontiguousarray(WV[h]).astype(BF),
            "wo": np.ascontiguousarray(WO).astype(BF),
            "bo": np.ascontiguousarray(bo_eff).astype(BF),
        })
    nc = _get_nc()
    res = run_bass_kernel_spmd(
        nc, in_maps, core_ids=list(range(N_CORES)),
        trace=_trace, **(_trace_kwargs or {}),
    )
    fullb = np.empty((B, S, D), dtype=np.float32)
    for c in range(N_CORES):
        oc = np.asarray(res.results[c]["out"], dtype=np.float32)
        b, off = c // 2, (c % 2) * QT
        for q in range(4):
            fullb[b, q * 512 + off:q * 512 + off + QT] = oc[q * QT:(q + 1) * QT]
    full = fullb.transpose(1, 0, 2)
    if _trace:
        return np.ascontiguousarray(full), res
    return np.ascontiguousarray(full)


# revision 30
# speedup vs baseline: 1.0108x; 1.0108x over previous
"""Trainium2 Bass kernel for nn_Encoder_78795470012907.

Encoder layer: per-head Q/K/V projections, scores = QK^T/sqrt(dk),
double softmax (over batch axis, then over key axis), Z = pV, concat
heads, output projection. S=2048, B=4, D=512, H=8, dk=dv=64.

Sharding: head-parallel over 8 cores (core h owns head h) for the
attention; four quarter-AllToAlls (one per s-chunk of 512) re-shard by
token for the output projection. Core c owns tokens
{(b=c//2, s = q*512 + (c%2)*256 + [0,256)) : q in 0..3} and emits them
as 4 quarters of 256 rows (host just scatters).

Layout notes (per core):
 - tokens are b-major: tok = b*2048 + s.
 - X is fed pre-transposed AND pre-cast from host as XT [D, NTOK] bf16.
 - V bias folds out (softmax rows sum to 1): bo' = bO + bv_cat @ WO.
 - phase A PSUM evacuation runs on the otherwise-idle DVE
   (tensor_scalar_add applies the Q/K bias per-partition); ACT does
   nothing in phase A so phase B's exp pipeline is the only ACT work.
 - scores are computed transposed ([t, s] tiles); the two batches of a
   b-pair run as concurrent row-tiled matmuls.
 - softmax over b: e=exp(s/8) -> D1 via two tree adds -> r=1/D1 on the
   custom-DVE fast reciprocal -> p1=e*r (all-bf16 multiply; a mixed
   bf16*f32 multiply drops DVE to the f32 rate, ~2x slower).
 - softmax over t rides the Z matmul via a ones-column appended to V
   (row 64 of the Z psum accumulates D2 = sum_t exp(p1)).
 - sc boundary: z rows evac on ACT Identity (DVE is the tighter engine
   in phase B), D2 row on DVE -- one PSUM bank released per b.  NO
   reciprocal/broadcast/multiply sits in the boundary chain (any
   broadcast there stalls the next sc's Z matmuls ~15us).
 - the a2a ships [65, 256] chunks (z + raw D2 row, bf16).  The dest
   normalizes at the TAIL: gpsimd partition_broadcast of the D2 row,
   cast-up on ACT, fast reciprocal + multiplies on DVE.
 - CRITICAL scheduling invariants, each worth 15-30us if violated:
   every collective-downstream op is gated behind a phase-B-end DVE
   memset (the static scheduler models collective latency
   optimistically and otherwise slots such ops mid-phase-B where they
   head-of-line block their engine queue); gather DMAs ride the sync
   queue only; DVE two-SBUF-input ops need equal base partitions;
   partition_broadcast needs a contiguous base-0 input.
"""

from contextlib import ExitStack

import numpy as np
import ml_dtypes

import concourse.bass as bass
import concourse.tile as tile
from concourse import bacc, mybir
from concourse.bass_utils import run_bass_kernel_spmd

S, B, D = 2048, 4, 512
H, DK, DV = 8, 64, 64
N_CORES = 8
NTOK = S * B          # 8192 tokens, b-major
TOKC = NTOK // N_CORES  # 1024 tokens per core for the output slice
SC = 512              # s-chunk (columns of a scores^T tile)
TC = 128              # t-chunk (partitions of a scores^T tile)
N_SC = S // SC        # 4
N_TC = S // TC        # 16
QT = 256              # tokens per (core, quarter)

F32 = mybir.dt.float32
BF16 = mybir.dt.bfloat16
AF = mybir.ActivationFunctionType
ALU = mybir.AluOpType
AX = mybir.AxisListType
BF = ml_dtypes.bfloat16


def build_kernel():
    nc = bacc.Bacc(num_devices=N_CORES)

    xt_d = nc.dram_tensor("xt", [D, NTOK], BF16, kind="ExternalInput")
    wqk_d = nc.dram_tensor("wqk", [D, 128], BF16, kind="ExternalInput")
    bqk_d = nc.dram_tensor("bqk", [128, 1], F32, kind="ExternalInput")
    wv_d = nc.dram_tensor("wv", [D, DV], BF16, kind="ExternalInput")
    wo_d = nc.dram_tensor("wo", [D, D], BF16, kind="ExternalInput")
    bo_d = nc.dram_tensor("bo", [1, D], BF16, kind="ExternalInput")
    out_d = nc.dram_tensor("out", [TOKC, D], BF16, kind="ExternalOutput")

    with tile.TileContext(nc) as tc, ExitStack() as ctx:
        pp = ctx.enter_context(tc.tile_pool(name="persist", bufs=1))
        dram = ctx.enter_context(tc.tile_pool(name="dram", bufs=1, space="DRAM"))

        # ---- persistent SBUF ----
        qt = [pp.tile([128, S], BF16, tag=f"qt{p}", name=f"qt{p}") for p in range(2)]
        kt = [pp.tile([128, S], BF16, tag=f"kt{p}", name=f"kt{p}") for p in range(2)]
        vt = pp.tile([128, 64 * 65], BF16, tag="vt", name="vt")
        # Z^T unnormalized [64, NTOK] + D2 row [1, NTOK], bf16
        zt = pp.tile([64, NTOK], BF16, tag="zt", name="zt")
        dz = pp.tile([1, NTOK], BF16, tag="dz", name="dz")

        wqk = [pp.tile([128, 128], BF16, tag=f"wqk{i}", name=f"wqk{i}") for i in range(4)]
        wv = [pp.tile([128, DV], BF16, tag=f"wv{i}", name=f"wv{i}") for i in range(4)]
        wo = [pp.tile([128, D], BF16, tag=f"wo{i}", name=f"wo{i}") for i in range(4)]
        bqk = pp.tile([128, 1], F32, tag="bqk", name="bqk")
        bo = pp.tile([1, D], BF16, tag="bo", name="bo")

        for i in range(4):
            nc.sync.dma_start(wqk[i][:], wqk_d[i * 128:(i + 1) * 128, :])
        nc.sync.dma_start(bqk[:], bqk_d[:])
        for i in range(4):
            nc.gpsimd.dma_start(wv[i][:], wv_d[i * 128:(i + 1) * 128, :])
            nc.gpsimd.dma_start(wo[i][:], wo_d[i * 128:(i + 1) * 128, :])
        nc.gpsimd.dma_start(bo[:], bo_d[:])

        onesf = pp.tile([128, 128], F32, tag="onesf", name="onesf")
        nc.vector.memset(onesf[:], 1.0)
        ones_bf = pp.tile([1, 128], BF16, tag="ones_bf", name="ones_bf")
        nc.vector.tensor_copy(ones_bf[:], onesf[0:1, :])
        vt_ones = vt[:].rearrange("p (n c) -> p n c", c=65)[:, :, 64:65]
        nc.vector.tensor_copy(vt_ones, onesf[:, 0:64, None])
        # warm the ACT exp table set and the gpsimd broadcast library
        warm = pp.tile([1, 1], BF16, tag="warm", name="warm")
        nc.scalar.activation(warm[:], onesf[0:1, 0:1], AF.Exp)
        warmb = pp.tile([2, 16], BF16, tag="warmb", name="warmb")
        nc.gpsimd.partition_broadcast(warmb[:], ones_bf[:, 0:16])

        # ================= Phase A: projections =================
        # PE-bound; PSUM evacuation (+ Q/K bias + bf16 cast) rides the
        # otherwise-idle DVE so ACT stays free for phase B's exps.
        with (
            tc.tile_pool(name="xtp", bufs=2) as xp,
            tc.tile_pool(name="psA", bufs=2, space="PSUM") as psA,
        ):
            for ssub in range(4):
                xtile = [xp.tile([128, 4, 512], BF16, tag=f"xt{i}",
                                 name=f"xtile{i}") for i in range(4)]
                xsrc = xt_d[:].rearrange("d (b ss s) -> d b ss s", b=4, ss=4)
                for i in range(4):
                    nc.sync.dma_start(
                        xtile[i][:],
                        xsrc[i * 128:(i + 1) * 128, :, ssub, :],
                    )
                for b in range(4):
                    pair, row = b // 2, (b % 2) * 64
                    pqk = psA.tile([128, 512], F32, tag="pqk", name="pqk",
                                   bufs=3)
                    for i in range(4):
                        nc.tensor.matmul(pqk[:], wqk[i][:], xtile[i][:, b, :],
                                         start=(i == 0), stop=(i == 3))
                    scol = ssub * 512
                    nc.vector.tensor_scalar_add(
                        qt[pair][row:row + 64, scol:scol + 512],
                        pqk[0:64, :], bqk[0:64, :])
                    nc.vector.tensor_scalar_add(
                        kt[pair][row:row + 64, scol:scol + 512],
                        pqk[64:128, :], bqk[64:128, :])
                    pv = psA.tile([128, 4 * DV], F32, tag="pv", name="pv",
                                  bufs=2)
                    for sub in range(4):
                        for i in range(4):
                            nc.tensor.matmul(
                                pv[:, sub * DV:(sub + 1) * DV],
                                xtile[i][:, b, sub * 128:(sub + 1) * 128],
                                wv[i][:], start=(i == 0), stop=(i == 3))
                    tbase = (b * 4 + ssub) * 4
                    nc.vector.tensor_copy(
                        vt[:].rearrange("p (n c) -> p n c", c=65)
                        [:, tbase:tbase + 4, 0:64],
                        pv[:].rearrange("p (n c) -> p n c", c=DV))

        # ================= Phase B: attention =================
        # per-quarter a2a staging: each quarter q ships, per dest core
        # c=(b,half), [65, 256]: 64 unnormalized z rows + the raw D2 row
        # for tokens (b, s in q*512 + half*256 + [0,256)).
        a2a_in_h = [dram.tile([N_CORES * 65, QT], BF16, tag=f"a2a_in{q}",
                              name=f"a2a_in{q}") for q in range(4)]
        a2a_out_h = [dram.tile([N_CORES * 65, QT], BF16, tag=f"a2a_out{q}",
                               name=f"a2a_out{q}") for q in range(4)]

        with (
            tc.tile_pool(name="wb", bufs=2) as wb,
        ):
            with tc.tile_pool(name="psB", bufs=1, space="PSUM") as psB:
                NB = N_SC * N_TC
                pipe = {}

                def softmax_b(g):
                    sc, t = g // N_TC, g % N_TC
                    scp = psB.tile([128, 4 * SC], F32, tag="scp", name="scp")
                    for pair in range(2):
                        for half in range(2):
                            b = pair * 2 + half
                            row = half * 64
                            nc.tensor.matmul(
                                scp[:, b * SC:(b + 1) * SC],
                                kt[pair][row:row + 64, t * TC:(t + 1) * TC],
                                qt[pair][row:row + 64, sc * SC:(sc + 1) * SC],
                                start=True, stop=True,
                            )
                    e = wb.tile([128, 4 * SC], BF16, tag="e", name="e", bufs=3)
                    nc.scalar.activation(e[:], scp[:], AF.Exp, scale=0.125)
                    t01 = wb.tile([128, 2 * SC], BF16, tag="t01", name="t01",
                                  bufs=2)
                    nc.vector.tensor_add(t01[:], e[:, 0:2 * SC],
                                         e[:, 2 * SC:4 * SC])
                    ddf = wb.tile([128, SC], F32, tag="ddf", name="ddf", bufs=2)
                    nc.vector.tensor_add(ddf[:], t01[:, 0:SC], t01[:, SC:2 * SC])
                    rf = wb.tile([128, SC], F32, tag="rf", name="rf", bufs=2)
                    nc.vector.reciprocal_approx_fast(rf[:], ddf[:])
                    rr = wb.tile([128, SC], BF16, tag="rr", name="rr", bufs=2)
                    nc.vector.tensor_copy(rr[:], rf[:])
                    p1 = wb.tile([128, 4 * SC], BF16, tag="p1", name="p1",
                                 bufs=3)
                    nc.vector.tensor_mul(
                        p1[:].rearrange("p (b s) -> p b s", b=4),
                        e[:].rearrange("p (b s) -> p b s", b=4),
                        rr[:, None, :].broadcast_to([128, 4, SC]),
                    )
                    pipe[g] = p1

                def exp2_and_z(g, zacc):
                    t = g % N_TC
                    p1 = pipe.pop(g)
                    q = wb.tile([128, 4 * SC], BF16, tag="q", name="q", bufs=3)
                    nc.scalar.activation(q[:], p1[:], AF.Exp)
                    for b in range(4):
                        tci = b * 16 + t
                        nc.tensor.matmul(
                            zacc[:, b * SC:(b + 1) * SC],
                            vt[:, tci * 65:(tci + 1) * 65],
                            q[:, b * SC:(b + 1) * SC],
                            start=(t == 0), stop=(t == N_TC - 1),
                        )

                def sc_boundary(sc_done, za):
                    # plain evac only: z rows -> zt (releases one PSUM bank
                    # per b), D2 row -> dz.  Normalization happens on the
                    # destination core at the tail.
                    with tc.high_priority():
                        ztv = zt[:].rearrange("p (b s8) -> p b s8", b=4)
                        dzv = dz[:].rearrange("p (b s8) -> p b s8", b=4)
                        for b in range(4):
                            sl = slice(b * SC, (b + 1) * SC)
                            ssl = slice(sc_done * SC, (sc_done + 1) * SC)
                            # z rows evac on ACT (rebalance: DVE is the
                            # tighter engine in phase B), D2 row on DVE
                            nc.scalar.activation(
                                ztv[:, b, ssl], za[0:64, sl], AF.Identity)
                            nc.vector.tensor_copy(
                                dzv[:, b, ssl], za[64:65, sl])
                        # ship quarter sc_done: z rows (2 DMAs by half) +
                        # D2 row (1 DMA)
                        zsrc = zt[:].rearrange(
                            "p (b sc half u) -> p b sc half u",
                            b=4, sc=4, half=2)
                        dsrc = dz[:].rearrange(
                            "p (b sc half u) -> p b sc half u",
                            b=4, sc=4, half=2)
                        av = a2a_in_h[sc_done][:].rearrange(
                            "(b half p) u -> p b half u", b=4, half=2, p=65)
                        for half in range(2):
                            nc.sync.dma_start(
                                av[0:64, :, half, :],
                                zsrc[:, :, sc_done, half, :])
                            nc.sync.dma_start(
                                av[64:65, :, half, :],
                                dsrc[:, :, sc_done, half, :])
                        nc.gpsimd.collective_compute(
                            "AllToAll",
                            ALU.bypass,
                            replica_groups=[list(range(N_CORES))],
                            ins=[a2a_in_h[sc_done][:].opt()],
                            outs=[a2a_out_h[sc_done][:].opt()],
                        )

                zaccs = {}
                for g in range(NB + 2):
                    if g < NB:
                        if g % N_TC == 0:
                            zaccs[g // N_TC] = psB.tile([65, 4 * SC], F32,
                                                        tag="zacc", name="zacc")
                        softmax_b(g)
                    if g >= 2:
                        gz = g - 2
                        za = zaccs[gz // N_TC]
                        exp2_and_z(gz, za)
                        if gz % N_TC == N_TC - 1:
                            sc_boundary(gz // N_TC, za)

            # ---- Phase C: normalize + output projection, per quarter ----
            # The static scheduler models collective latency optimistically
            # and will happily slot collective-downstream ops into
            # mid-phase-B queue positions, where they head-of-line block
            # the engine queues (~20us each).  GATE each quarter's gathers
            # behind a tiny DVE memset emitted at the END of phase B so no
            # dest-side op can be scheduled before phase B drains.
            with (
                tc.tile_pool(name="psC", bufs=2, space="PSUM") as psC,
                tc.tile_pool(name="oc", bufs=1) as oc,
            ):
                zcs = [oc.tile([64, 8 * QT], BF16, tag=f"zc{q}",
                               name=f"zc{q}") for q in range(4)]
                dqs = [oc.tile([1, 8 * QT], BF16, tag=f"dq{q}",
                               name=f"dq{q}") for q in range(4)]
                for q in range(4):
                    nc.vector.memset(zcs[q][0:1, :], 0.0)
                    nc.vector.memset(dqs[q][:], 0.0)
                for q in range(4):
                    zc, dq = zcs[q], dqs[q]
                    src = a2a_out_h[q][:].rearrange(
                        "(h p) u -> p h u", p=65)
                    # gathers ride the sync queue (nothing queued behind
                    # them at the tail can matter)
                    nc.sync.dma_start(
                        zc[:].rearrange("p (h u) -> p h u", h=8),
                        src[0:64, :, :])
                    nc.sync.dma_start(
                        dq[:].rearrange("p (h u) -> p h u", h=8),
                        src[64:65, :, :])
                    # dest-side normalize: ONE base-0 partition_broadcast of
                    # the raw D2 row, cast-up on ACT, 1/x on DVE, then two
                    # strided equal-base multiplies into the head-pair
                    # stacked zcn layout phase C consumes.
                    rb = oc.tile([64, 8 * QT], BF16, tag="rb", name="rb",
                                 bufs=2)
                    nc.gpsimd.partition_broadcast(rb[:], dq[:])
                    rbf = oc.tile([64, 8 * QT], F32, tag="rbf", name="rbf",
                                  bufs=2)
                    nc.scalar.activation(rbf[:], rb[:], AF.Identity)
                    rbr = oc.tile([64, 8 * QT], F32, tag="rbr", name="rbr",
                                  bufs=2)
                    nc.vector.reciprocal_approx_fast(rbr[:], rbf[:])
                    zcn = oc.tile([128, 4 * QT], BF16, tag="zcn", name="zcn",
                                  bufs=2)
                    zcv = zc[:].rearrange("p (i hh u) -> p hh i u",
                                          i=4, hh=2)
                    rbv = rbr[:].rearrange("p (i hh u) -> p hh i u",
                                           i=4, hh=2)
                    for hh in range(2):
                        nc.vector.tensor_mul(
                            zcn[hh * 64:(hh + 1) * 64, :]
                            .rearrange("p (i u) -> p i u", i=4),
                            zcv[:, hh, :, :], rbv[:, hh, :, :])
                    for m in range(QT // 128):
                        po = psC.tile([128, D], F32, tag="po", name="po")
                        for i in range(4):
                            nc.tensor.matmul(po[:],
                                             zcn[:, i * QT + m * 128:i * QT + (m + 1) * 128],
                                             wo[i][:], start=(i == 0),
                                             stop=False)
                        nc.tensor.matmul(po[:], ones_bf[:], bo[:], start=False,
                                         stop=True)
                        ot = oc.tile([128, D], BF16, tag="ot", name="ot")
                        nc.scalar.activation(ot[:], po[:], AF.Identity)
                        row = q * QT + m * 128
                        # out DMAs follow their evacs on the scalar queue
                        nc.scalar.dma_start(out_d[row:row + 128, :], ot[:])

    nc.compile()
    return nc


_NC_CACHE = None


def _get_nc():
    global _NC_CACHE
    if _NC_CACHE is None:
        _NC_CACHE = build_kernel()
    return _NC_CACHE


def kernel(X, WQ, bQ, WK, bK, WV, bV, WO, bO, _trace=False, _trace_kwargs=None):
    """Full inputs in, full output out. Shards internally across 8 cores."""
    X = np.asarray(X, dtype=np.float32)
    WQ, bQ = np.asarray(WQ, np.float32), np.asarray(bQ, np.float32)
    WK, bK = np.asarray(WK, np.float32), np.asarray(bK, np.float32)
    WV, bV = np.asarray(WV, np.float32), np.asarray(bV, np.float32)
    WO, bO = np.asarray(WO, np.float32), np.asarray(bO, np.float32)
    xt = np.ascontiguousarray(
        X.transpose(2, 1, 0).reshape(D, NTOK)).astype(BF)
    # fold the V bias through the output projection (softmax rows sum to 1)
    bo_eff = (bO + bV.reshape(1, H * DV) @ WO).astype(np.float32)
    in_maps = []
    for h in range(N_CORES):
        wqk = np.ascontiguousarray(
            np.concatenate([WQ[h], WK[h]], axis=1)).astype(BF)
        bqk = np.ascontiguousarray(
            np.concatenate([bQ[h], bK[h]])[:, None], dtype=np.float32)
        in_maps.append({
            "xt": xt,
            "wqk": wqk,
            "bqk": bqk,
            "wv": np.ascontiguousarray(WV[h]).astype(BF),
            "wo": np.ascontiguousarray(WO).astype(BF),
            "bo": np.ascontiguousarray(bo_eff).astype(BF),
        })
    nc = _get_nc()
    res = run_bass_kernel_spmd(
        nc, in_maps, core_ids=list(range(N_CORES)),
        trace=_trace, **(_trace_kwargs or {}),
    )
    fullb = np.empty((B, S, D), dtype=np.float32)
    for c in range(N_CORES):
        oc = np.asarray(res.results[c]["out"], dtype=np.float32)
        b, off = c // 2, (c % 2) * QT
        for q in range(4):
            fullb[b, q * 512 + off:q * 512 + off + QT] = oc[q * QT:(q + 1) * QT]
    full = fullb.transpose(1, 0, 2)
    if _trace:
        return np.ascontiguousarray(full), res
    return np.ascontiguousarray(full)
